# revision 5
# baseline (speedup 1.0000x reference)
"""Bahdanau attention kernel for Trainium2 (8 NeuronCores, data-parallel over batch).

Reference computation (B=32, T=4096, D=U=512):
    q_proj = query @ W1 + b1                      [B, 1, U]
    v_proj = values @ W2 + b2                     [B, T, U]
    scores = tanh(q_proj + v_proj) @ V + bv       [B, T, 1]
    attn   = softmax(scores, axis=1)
    out    = sum(attn * values, axis=1)           [B, D]

Device strategy (per core, 4 batches), using only PE + ACT + DMA:
  - Host folds b1/b2 into q_eff = query@W1 + b1 + b2, drops bv (softmax shift
    invariance), ships values twice: natural [T, D] bf16 (context matmul) and
    transposed [D, T] fp8 (projection matmul). All DRAM tensors are
    pre-swizzled on the host into the exact SBUF tile layout so every DMA
    reads large contiguous per-partition segments (max-size packets).
  - v_proj computed transposed [U, t] with W2 stationary; fp8 DoubleRow
    (2 matmuls of K=256) with W2 pre-scaled by F8_SCALE on host, un-scaled
    inside the ACT tanh (scale=1/F8_SCALE); q_eff rides the tanh bias.
  - scores: tanh tiles become the stationary operand against V [128,1], so
    scores land directly in [128, T/128] partition-major PSUM layout. The
    stationary loads run at FWL 4x rate, ~25ns per 128-col tile.
  - softmax without division or max-subtraction (|scores| <= ~1.3 here):
    attn = exp(s); normalization by sum happens on the host via colsums.
  - Context: 4-way column-tiled accumulating [128,1]x[128,512] matmuls:
    t-block n goes to PE column-group n%4 (tile_position=(0,32*(n%4))),
    4 matmuls stream concurrently through disjoint column strips. Partial
    contexts land on PSUM partitions {0,32,64,96}; host sums the 4 rows.
"""

import os
import sys

import numpy as np

try:
    import ml_dtypes  # noqa: F401
except ImportError:  # pragma: no cover
    sys.path.insert(0, "/opt/trn_rl_repo")
    import ml_dtypes  # noqa: F401

try:
    import concourse  # noqa: F401
except ImportError:  # pragma: no cover
    sys.path.insert(0, "/opt/trn_rl_repo")

BF16 = np.dtype(ml_dtypes.bfloat16)
FP8 = np.dtype(ml_dtypes.float8_e4m3)

B, T, D, U = 32, 4096, 512, 512
N_CORES = 8
BPC = B // N_CORES  # batches per core

F8_SCALE = 64.0  # host scales W2 by this; ACT tanh un-scales via scale=1/F8_SCALE

MODE = os.environ.get("BAHDANAU_MODE", "fp8")  # "fp8" | "bf16"

_MODULES: dict = {}


def _build(bpc: int = BPC, t: int = T, mode: str = "fp8"):
    """Build + compile the per-core Bass module. Shapes are per-core shards."""
    from contextlib import ExitStack

    import concourse.bass as bass
    import concourse.tile as tile
    from concourse import bacc, mybir

    f32 = mybir.dt.float32
    bf16 = mybir.dt.bfloat16
    fp8 = mybir.dt.float8e4
    FT = mybir.ActivationFunctionType
    PSUM = bass.MemorySpace.PSUM
    DR = mybir.MatmulPerfMode.DoubleRow

    use_fp8 = mode == "fp8"
    vt_dt = fp8 if use_fp8 else bf16
    tb_n = t // 128  # 128-row t-blocks per batch (32)
    tc_n = t // 512  # 512-col t-chunks per batch (8)
    ch_n = t // 1024  # 1024-col DMA chunks per batch (4)
    tanh_scale = (1.0 / F8_SCALE) if use_fp8 else 1.0

    nc = bacc.Bacc(
        "TRN2", target_bir_lowering=False, debug=False, enable_asserts=False
    )

    # All DRAM layouts match the SBUF tile layouts exactly (host pre-swizzles)
    vT_d = nc.dram_tensor("valuesT", [bpc, ch_n, 128, 4, 1024], vt_dt,
                          kind="ExternalInput")
    vN_d = nc.dram_tensor("valuesN", [bpc, 4, 128, 8, 512], bf16,
                          kind="ExternalInput")
    w2_d = nc.dram_tensor("w2t", [128, 4, U], vt_dt, kind="ExternalInput")
    vc_d = nc.dram_tensor("v_col", [128, 4], bf16, kind="ExternalInput")
    qe_d = nc.dram_tensor("q_eff", [128, bpc, 4], f32, kind="ExternalInput")
    c1b_d = nc.dram_tensor("c_ones_bf", [128, 1], bf16, kind="ExternalInput")
    out_d = nc.dram_tensor("ctx_out", [bpc, 4, D], f32, kind="ExternalOutput")
    cols_d = nc.dram_tensor("colsums", [bpc, tb_n], f32, kind="ExternalOutput")

    with tile.TileContext(nc) as tc, ExitStack() as ctx:
        const = ctx.enter_context(tc.tile_pool(name="const", bufs=1))
        vT_pool = ctx.enter_context(tc.tile_pool(name="vT", bufs=3))
        vN_pool = ctx.enter_context(tc.tile_pool(name="vN", bufs=3))
        tanh_pool = ctx.enter_context(tc.tile_pool(name="tanh", bufs=8))
        sm_pool = ctx.enter_context(tc.tile_pool(name="sm", bufs=2))
        attn_pool = ctx.enter_context(tc.tile_pool(name="attn", bufs=2))
        ctxs_pool = ctx.enter_context(tc.tile_pool(name="ctxs", bufs=2))
        vp_psum = ctx.enter_context(tc.tile_pool(name="vp_ps", bufs=2, space=PSUM))
        sco_psum = ctx.enter_context(tc.tile_pool(name="sc_ps", bufs=2, space=PSUM))
        ctx_psum = ctx.enter_context(tc.tile_pool(name="ctx_ps", bufs=1, space=PSUM))
        sms_psum = ctx.enter_context(tc.tile_pool(name="sm_ps", bufs=1, space=PSUM))

        w2_sb = const.tile([128, 4, U], vt_dt)
        nc.sync.dma_start(w2_sb[:], w2_d.ap())
        vc_sb = const.tile([128, 4], bf16)
        nc.sync.dma_start(vc_sb[:], vc_d.ap())
        qe_sb = const.tile([128, bpc, 4], f32)
        nc.sync.dma_start(qe_sb[:], qe_d.ap())
        c1b_sb = const.tile([128, 1], bf16)
        nc.sync.dma_start(c1b_sb[:], c1b_d.ap())

        def stage(b, prev_tail):
            """Full per-batch pipeline: load, project, scores, exp, context.

            The previous batch's final-pair context/sum work (`prev_tail`) is
            emitted after this batch's first pair so the PE never stalls the
            ACT pipeline at batch boundaries. Returns this batch's tail."""
            n_pairs = tc_n // 2
            vT_sb = vT_pool.tile([128, ch_n, 4, 1024], vt_dt)
            for c in range(ch_n):
                nc.sync.dma_start(vT_sb[:, c], vT_d[b, c])
            vN_sb = vN_pool.tile([128, 4, 8, D], bf16)
            scoresP = sco_psum.tile([128, tb_n], f32)
            expP = attn_pool.tile([128, tb_n], bf16)
            cps = ctx_psum.tile([128, D], f32)

            def ctx_mms(pair):
                # 4-way column-tiled accumulation: t-block n -> col group n%4,
                # 4 concurrent matmuls in disjoint column strips. Each group's
                # first matmul carries start=True (clears has_written for its
                # own partition region) and its last carries stop=True.
                for k in range(8):
                    n = pair * 8 + k
                    g = n % 4
                    nc.tensor.matmul(
                        cps[32 * g : 32 * g + 1, :],
                        expP[:, n : n + 1],
                        vN_sb[:, pair, k, :],
                        start=(n < 4),
                        stop=(n >= tb_n - 4),
                        tile_position=(0, 32 * g),
                        skip_group_check=True,
                    )

            for pair in range(n_pairs):
                nc.sync.dma_start(vN_sb[:, pair], vN_d[b, pair])
                tanh_tiles = []
                for ub in range(4):
                    vp = vp_psum.tile([128, 2, 512], f32)
                    # j outer / half inner so consecutive matmuls share the
                    # same stationary W2 block (LDWEIGHTS amortization)
                    if use_fp8:
                        for j in range(2):
                            for half in range(2):
                                tc8 = pair * 2 + half
                                nc.tensor.matmul(
                                    vp[:, half, :],
                                    w2_sb[:, 2 * j : 2 * j + 2, bass.ts(ub, 128)],
                                    vT_sb[:, tc8 // 2, 2 * j : 2 * j + 2,
                                          bass.ts(tc8 % 2, 512)],
                                    start=(j == 0),
                                    stop=(j == 1),
                                    perf_mode=DR,
                                )
                    else:
                        for j in range(4):
                            for half in range(2):
                                tc8 = pair * 2 + half
                                nc.tensor.matmul(
                                    vp[:, half, :],
                                    w2_sb[:, j, bass.ts(ub, 128)],
                                    vT_sb[:, tc8 // 2, j, bass.ts(tc8 % 2, 512)],
                                    start=(j == 0),
                                    stop=(j == 3),
                                )
                    th = tanh_pool.tile([128, 2, 512], bf16)
                    nc.scalar.activation(
                        th[:],
                        vp[:],
                        FT.Tanh,
                        bias=qe_sb[:, b, ub : ub + 1],
                        scale=tanh_scale,
                    )
                    tanh_tiles.append(th)
                for tl8 in range(8):
                    blk = pair * 8 + tl8
                    for ub in range(4):
                        nc.tensor.matmul(
                            scoresP[:, blk : blk + 1],
                            tanh_tiles[ub][:, tl8 // 4, bass.ts(tl8 % 4, 128)],
                            vc_sb[:, ub : ub + 1],
                            start=(ub == 0),
                            stop=(ub == 3),
                        )
                # unnormalized softmax weights for this pair; division by the
                # sum of exp happens on the host via the colsums output
                nc.scalar.activation(
                    expP[:, pair * 8 : (pair + 1) * 8],
                    scoresP[:, pair * 8 : (pair + 1) * 8],
                    FT.Exp,
                )
                if pair == 0 and prev_tail is not None:
                    prev_tail()
                if pair > 0:
                    ctx_mms(pair - 1)

            def tail():
                ctx_mms(n_pairs - 1)
                ps1 = sms_psum.tile([tb_n, 1], f32, tag="sm")
                nc.tensor.matmul(
                    ps1[:], expP[:], c1b_sb[:], start=True, stop=True
                )
                s1 = sm_pool.tile([tb_n, 1], f32, tag="s1")
                nc.scalar.copy(s1[:], ps1[:])
                nc.sync.dma_start(
                    cols_d[b : b + 1, :].rearrange("one n -> n one"), s1[:]
                )
                cs_raw = ctxs_pool.tile([128, D], f32)
                nc.scalar.copy(cs_raw[:], cps[:])
                nc.sync.dma_start(
                    out_d[b],
                    cs_raw.rearrange("(g s) d -> g s d", s=32)[:, 0, :],
                )

            return tail

        pend = None
        for b in range(bpc):
            pend = stage(b, pend)
        pend()

    nc.compile()
    return nc


def _get_module(bpc: int = BPC, t: int = T, mode: str | None = None):
    mode = MODE if mode is None else mode
    key = (mode, bpc, t)
    if key not in _MODULES:
        _MODULES[key] = _build(bpc, t, mode)
    return _MODULES[key]


def _prep_inputs(query, values, W1, b1, W2, b2, V, bv, mode: str | None = None):
    """Host-side preprocessing: fold biases, cast, transpose, swizzle, shard."""
    mode = MODE if mode is None else mode
    query = np.asarray(query, np.float32)
    values = np.asarray(values, np.float32)
    W1 = np.asarray(W1, np.float32)
    b1 = np.asarray(b1, np.float32)
    W2 = np.asarray(W2, np.float32)
    b2 = np.asarray(b2, np.float32)
    V = np.asarray(V, np.float32)

    q_eff = (
        query.astype(np.float64) @ W1.astype(np.float64)
        + b1.astype(np.float64)
        + b2.astype(np.float64)
    ).astype(np.float32)  # [B, U]; bv dropped (softmax shift invariance)

    # values natural [T, D] -> [4(pair), 128(p), 8(blk), 512] per batch
    vN = np.ascontiguousarray(
        values.reshape(B, 4, 8, 128, D).transpose(0, 1, 3, 2, 4)
    ).astype(BF16)
    # values transposed [D, T] -> [4(chunk), 128(p), 4(db), 1024] per batch
    vTf = values.transpose(0, 2, 1).reshape(B, 4, 128, 4, 1024)
    vTf = np.ascontiguousarray(vTf.transpose(0, 3, 2, 1, 4))
    if mode == "fp8":
        vT = vTf.astype(FP8)
        w2 = (W2 * F8_SCALE).astype(FP8)
    else:
        vT = vTf.astype(BF16)
        w2 = W2.astype(BF16)
    # W2 [D, U] -> [128(p), 4(db), U]
    w2 = np.ascontiguousarray(w2.reshape(4, 128, U).transpose(1, 0, 2))
    # V [U, 1] -> [128(p), 4(ub)]
    vcol = np.ascontiguousarray(V.reshape(4, 128).T).astype(BF16)
    # q_eff [B, U] -> per-core [128(p), bpc, 4(ub)]
    qe = np.ascontiguousarray(
        q_eff.reshape(B, 4, 128).transpose(2, 0, 1)
    )
    c1b = np.ones((128, 1), BF16)

    in_maps = []
    for c in range(N_CORES):
        s = slice(c * BPC, (c + 1) * BPC)
        in_maps.append(
            {
                "valuesT": vT[s],
                "valuesN": vN[s],
                "w2t": w2,
                "v_col": vcol,
                "q_eff": qe[:, s],
                "c_ones_bf": c1b,
            }
        )
    return in_maps


def _run(in_maps, trace=False, mode: str | None = None, **kw):
    from concourse.bass_utils import run_bass_kernel_spmd

    nc = _get_module(mode=mode)
    res = run_bass_kernel_spmd(
        nc, in_maps, core_ids=list(range(N_CORES)), trace=trace, **kw
    )
    raw = np.concatenate(
        [np.asarray(res.results[c]["ctx_out"]) for c in range(N_CORES)], axis=0
    ).astype(np.float32)  # [B, 4, D] partial contexts per column group
    sums = np.concatenate(
        [np.asarray(res.results[c]["colsums"]) for c in range(N_CORES)], axis=0
    ).astype(np.float32)
    out = raw.sum(axis=1) / sums.sum(axis=1, keepdims=True)
    return out, res


def kernel(query, values, W1, b1, W2, b2, V, bv):
    in_maps = _prep_inputs(query, values, W1, b1, W2, b2, V, bv)
    out, _ = _run(in_maps, trace=False)
    return out


# revision 8
# speedup vs baseline: 1.2098x; 1.2098x over previous
"""Bahdanau attention kernel for Trainium2 (8 NeuronCores, data-parallel over batch).

Reference computation (B=32, T=4096, D=U=512):
    q_proj = query @ W1 + b1                      [B, 1, U]
    v_proj = values @ W2 + b2                     [B, T, U]
    scores = tanh(q_proj + v_proj) @ V + bv       [B, T, 1]
    attn   = softmax(scores, axis=1)
    out    = sum(attn * values, axis=1)           [B, D]

Device strategy (per core, 4 batches), using only PE + ACT + DMA:
  - Host folds b1/b2 into q_eff = query@W1 + b1 + b2, drops bv (softmax shift
    invariance), ships values twice: natural [T, D] bf16 (context matmul) and
    transposed [D, T] fp8 (projection matmul). All DRAM tensors are
    pre-swizzled on the host into the exact SBUF tile layout so every DMA
    reads large contiguous per-partition segments (max-size packets).
  - v_proj computed transposed [U, t] with W2 stationary; fp8 DoubleRow
    (2 matmuls of K=256) with W2 pre-scaled by F8_SCALE on host, un-scaled
    inside the ACT tanh (scale=1/F8_SCALE); q_eff rides the tanh bias.
  - scores: tanh tiles become the stationary operand against V [128,1], so
    scores land directly in [128, T/128] partition-major PSUM layout. The
    stationary loads run at FWL 4x rate, ~25ns per 128-col tile.
  - softmax without division or max-subtraction (|scores| <= ~1.3 here):
    attn = exp(s); normalization by sum happens on the host via colsums.
  - Context: 4-way column-tiled accumulating [128,1]x[128,512] matmuls:
    t-block n goes to PE column-group n%4 (tile_position=(0,32*(n%4))),
    4 matmuls stream concurrently through disjoint column strips. Partial
    contexts land on PSUM partitions {0,32,64,96}; host sums the 4 rows.
"""

import os
import sys

import numpy as np

try:
    import ml_dtypes  # noqa: F401
except ImportError:  # pragma: no cover
    sys.path.insert(0, "/opt/trn_rl_repo")
    import ml_dtypes  # noqa: F401

try:
    import concourse  # noqa: F401
except ImportError:  # pragma: no cover
    sys.path.insert(0, "/opt/trn_rl_repo")

BF16 = np.dtype(ml_dtypes.bfloat16)
FP8 = np.dtype(ml_dtypes.float8_e4m3)

B, T, D, U = 32, 4096, 512, 512
N_CORES = 8
BPC = B // N_CORES  # batches per core

F8_SCALE = 64.0  # host scales W2 by this; ACT tanh un-scales via scale=1/F8_SCALE

MODE = os.environ.get("BAHDANAU_MODE", "fp8")  # "fp8" | "bf16"

_MODULES: dict = {}


def _build(bpc: int = BPC, t: int = T, mode: str = "fp8"):
    """Build + compile the per-core Bass module. Shapes are per-core shards."""
    from contextlib import ExitStack

    import concourse.bass as bass
    import concourse.tile as tile
    from concourse import bacc, mybir

    f32 = mybir.dt.float32
    bf16 = mybir.dt.bfloat16
    fp8 = mybir.dt.float8e4
    FT = mybir.ActivationFunctionType
    PSUM = bass.MemorySpace.PSUM
    DR = mybir.MatmulPerfMode.DoubleRow

    use_fp8 = mode == "fp8"
    vt_dt = fp8 if use_fp8 else bf16
    tb_n = t // 128  # 128-row t-blocks per batch (32)
    tc_n = t // 512  # 512-col t-chunks per batch (8)
    ch_n = t // 1024  # 1024-col DMA chunks per batch (4)
    tanh_scale = (1.0 / F8_SCALE) if use_fp8 else 1.0

    nc = bacc.Bacc(
        "TRN2", target_bir_lowering=False, debug=False, enable_asserts=False
    )

    # All DRAM layouts match the SBUF tile layouts exactly (host pre-swizzles)
    vT_d = nc.dram_tensor("valuesT", [bpc, ch_n, 128, 4, 1024], vt_dt,
                          kind="ExternalInput")
    vN_d = nc.dram_tensor("valuesN", [bpc, 4, 128, 8, 512], bf16,
                          kind="ExternalInput")
    w2_d = nc.dram_tensor("w2t", [128, 4, U], vt_dt, kind="ExternalInput")
    vc_d = nc.dram_tensor("v_col", [128, 4], bf16, kind="ExternalInput")
    qe_d = nc.dram_tensor("q_eff", [128, bpc, 4], f32, kind="ExternalInput")
    c1b_d = nc.dram_tensor("c_ones_bf", [128, 1], bf16, kind="ExternalInput")
    out_d = nc.dram_tensor("ctx_out", [bpc, 4, D], f32, kind="ExternalOutput")
    cols_d = nc.dram_tensor("colsums", [bpc, tb_n], f32, kind="ExternalOutput")

    with tile.TileContext(nc) as tc, ExitStack() as ctx:
        const = ctx.enter_context(tc.tile_pool(name="const", bufs=1))
        vT_pool = ctx.enter_context(tc.tile_pool(name="vT", bufs=3))
        vN_pool = ctx.enter_context(tc.tile_pool(name="vN", bufs=3))
        tanh_pool = ctx.enter_context(tc.tile_pool(name="tanh", bufs=8))
        sm_pool = ctx.enter_context(tc.tile_pool(name="sm", bufs=2))
        attn_pool = ctx.enter_context(tc.tile_pool(name="attn", bufs=2))
        ctxs_pool = ctx.enter_context(tc.tile_pool(name="ctxs", bufs=2))
        vp_psum = ctx.enter_context(tc.tile_pool(name="vp_ps", bufs=2, space=PSUM))
        sco_psum = ctx.enter_context(tc.tile_pool(name="sc_ps", bufs=2, space=PSUM))
        ctx_psum = ctx.enter_context(tc.tile_pool(name="ctx_ps", bufs=1, space=PSUM))
        sms_psum = ctx.enter_context(tc.tile_pool(name="sm_ps", bufs=1, space=PSUM))

        # Startup order matters: w2 + the first vT half-chunk gate the first
        # projection matmul, qe gates the first tanh. Everything else follows.
        w2_sb = const.tile([128, 4, U], vt_dt)
        nc.sync.dma_start(w2_sb[:], w2_d.ap())
        vT0_sb = vT_pool.tile([128, ch_n, 4, 1024], vt_dt, tag="vT")
        nc.sync.dma_start(vT0_sb[:, 0, :, :512], vT_d[0, 0][:, :, :512])
        qe_sb = const.tile([128, bpc, 4], f32)
        nc.sync.dma_start(qe_sb[:], qe_d.ap())
        nc.sync.dma_start(vT0_sb[:, 0, :, 512:], vT_d[0, 0][:, :, 512:])
        vc_sb = const.tile([128, 4], bf16)
        nc.sync.dma_start(vc_sb[:], vc_d.ap())
        c1b_sb = const.tile([128, 1], bf16)
        nc.sync.dma_start(c1b_sb[:], c1b_d.ap())

        def stage(b, prev_tail):
            """Full per-batch pipeline: load, project, scores, exp, context.

            The previous batch's final-pair context/sum work (`prev_tail`) is
            emitted after this batch's first pair so the PE never stalls the
            ACT pipeline at batch boundaries. Returns this batch's tail."""
            n_pairs = tc_n // 2
            if b == 0:
                vT_sb = vT0_sb  # chunk 0 already in flight from the preamble
                for c in range(1, ch_n):
                    nc.sync.dma_start(vT_sb[:, c], vT_d[b, c])
            else:
                vT_sb = vT_pool.tile([128, ch_n, 4, 1024], vt_dt, tag="vT")
                for c in range(ch_n):
                    nc.sync.dma_start(vT_sb[:, c], vT_d[b, c])
            vN_sb = vN_pool.tile([128, 4, 8, D], bf16)
            scoresP = sco_psum.tile([128, tb_n], f32)
            expP = attn_pool.tile([128, tb_n], bf16)
            cps = ctx_psum.tile([128, D], f32)

            def ctx_mms(pair):
                # 4-way column-tiled accumulation: t-block n -> col group n%4,
                # 4 concurrent matmuls in disjoint column strips. Each group's
                # first matmul carries start=True (clears has_written for its
                # own partition region) and its last carries stop=True.
                for k in range(8):
                    n = pair * 8 + k
                    g = n % 4
                    nc.tensor.matmul(
                        cps[32 * g : 32 * g + 1, :],
                        expP[:, n : n + 1],
                        vN_sb[:, pair, k, :],
                        start=(n < 4),
                        stop=(n >= tb_n - 4),
                        tile_position=(0, 32 * g),
                        skip_group_check=True,
                    )

            for pair in range(n_pairs):
                nc.sync.dma_start(vN_sb[:, pair], vN_d[b, pair])
                tanh_tiles = []
                for ub in range(4):
                    vp = vp_psum.tile([128, 2, 512], f32)
                    # j outer / half inner so consecutive matmuls share the
                    # same stationary W2 block (LDWEIGHTS amortization)
                    if use_fp8:
                        for j in range(2):
                            for half in range(2):
                                tc8 = pair * 2 + half
                                nc.tensor.matmul(
                                    vp[:, half, :],
                                    w2_sb[:, 2 * j : 2 * j + 2, bass.ts(ub, 128)],
                                    vT_sb[:, tc8 // 2, 2 * j : 2 * j + 2,
                                          bass.ts(tc8 % 2, 512)],
                                    start=(j == 0),
                                    stop=(j == 1),
                                    perf_mode=DR,
                                )
                    else:
                        for j in range(4):
                            for half in range(2):
                                tc8 = pair * 2 + half
                                nc.tensor.matmul(
                                    vp[:, half, :],
                                    w2_sb[:, j, bass.ts(ub, 128)],
                                    vT_sb[:, tc8 // 2, j, bass.ts(tc8 % 2, 512)],
                                    start=(j == 0),
                                    stop=(j == 3),
                                )
                    th = tanh_pool.tile([128, 2, 512], bf16)
                    nc.scalar.activation(
                        th[:],
                        vp[:],
                        FT.Tanh,
                        bias=qe_sb[:, b, ub : ub + 1],
                        scale=tanh_scale,
                    )
                    tanh_tiles.append(th)
                for tl8 in range(8):
                    blk = pair * 8 + tl8
                    for ub in range(4):
                        nc.tensor.matmul(
                            scoresP[:, blk : blk + 1],
                            tanh_tiles[ub][:, tl8 // 4, bass.ts(tl8 % 4, 128)],
                            vc_sb[:, ub : ub + 1],
                            start=(ub == 0),
                            stop=(ub == 3),
                        )
                # unnormalized softmax weights for this pair; division by the
                # sum of exp happens on the host via the colsums output
                nc.scalar.activation(
                    expP[:, pair * 8 : (pair + 1) * 8],
                    scoresP[:, pair * 8 : (pair + 1) * 8],
                    FT.Exp,
                )
                if pair == 0 and prev_tail is not None:
                    prev_tail()
                if pair > 0:
                    ctx_mms(pair - 1)

            def tail():
                ctx_mms(n_pairs - 1)
                ps1 = sms_psum.tile([tb_n, 1], f32, tag="sm")
                nc.tensor.matmul(
                    ps1[:], expP[:], c1b_sb[:], start=True, stop=True
                )
                s1 = sm_pool.tile([tb_n, 1], f32, tag="s1")
                nc.vector.tensor_copy(s1[:], ps1[:])
                nc.sync.dma_start(
                    cols_d[b : b + 1, :].rearrange("one n -> n one"), s1[:]
                )
                cs_raw = ctxs_pool.tile([128, D], f32)
                nc.vector.tensor_copy(cs_raw[:], cps[:])
                nc.sync.dma_start(
                    out_d[b],
                    cs_raw.rearrange("(g s) d -> g s d", s=32)[:, 0, :],
                )

            return tail

        pend = None
        for b in range(bpc):
            pend = stage(b, pend)
        pend()

    nc.compile()
    return nc


def _get_module(bpc: int = BPC, t: int = T, mode: str | None = None):
    mode = MODE if mode is None else mode
    key = (mode, bpc, t)
    if key not in _MODULES:
        _MODULES[key] = _build(bpc, t, mode)
    return _MODULES[key]


def _prep_inputs(query, values, W1, b1, W2, b2, V, bv, mode: str | None = None):
    """Host-side preprocessing: fold biases, cast, transpose, swizzle, shard."""
    mode = MODE if mode is None else mode
    query = np.asarray(query, np.float32)
    values = np.asarray(values, np.float32)
    W1 = np.asarray(W1, np.float32)
    b1 = np.asarray(b1, np.float32)
    W2 = np.asarray(W2, np.float32)
    b2 = np.asarray(b2, np.float32)
    V = np.asarray(V, np.float32)

    q_eff = (
        query.astype(np.float64) @ W1.astype(np.float64)
        + b1.astype(np.float64)
        + b2.astype(np.float64)
    ).astype(np.float32)  # [B, U]; bv dropped (softmax shift invariance)

    # values natural [T, D] -> [4(pair), 128(p), 8(blk), 512] per batch
    vN = np.ascontiguousarray(
        values.reshape(B, 4, 8, 128, D).transpose(0, 1, 3, 2, 4)
    ).astype(BF16)
    # values transposed [D, T] -> [4(chunk), 128(p), 4(db), 1024] per batch
    vTf = values.transpose(0, 2, 1).reshape(B, 4, 128, 4, 1024)
    vTf = np.ascontiguousarray(vTf.transpose(0, 3, 2, 1, 4))
    if mode == "fp8":
        vT = vTf.astype(FP8)
        w2 = (W2 * F8_SCALE).astype(FP8)
    else:
        vT = vTf.astype(BF16)
        w2 = W2.astype(BF16)
    # W2 [D, U] -> [128(p), 4(db), U]
    w2 = np.ascontiguousarray(w2.reshape(4, 128, U).transpose(1, 0, 2))
    # V [U, 1] -> [128(p), 4(ub)]
    vcol = np.ascontiguousarray(V.reshape(4, 128).T).astype(BF16)
    # q_eff [B, U] -> per-core [128(p), bpc, 4(ub)]
    qe = np.ascontiguousarray(
        q_eff.reshape(B, 4, 128).transpose(2, 0, 1)
    )
    c1b = np.ones((128, 1), BF16)

    in_maps = []
    for c in range(N_CORES):
        s = slice(c * BPC, (c + 1) * BPC)
        in_maps.append(
            {
                "valuesT": vT[s],
                "valuesN": vN[s],
                "w2t": w2,
                "v_col": vcol,
                "q_eff": qe[:, s],
                "c_ones_bf": c1b,
            }
        )
    return in_maps


def _run(in_maps, trace=False, mode: str | None = None, **kw):
    from concourse.bass_utils import run_bass_kernel_spmd

    nc = _get_module(mode=mode)
    res = run_bass_kernel_spmd(
        nc, in_maps, core_ids=list(range(N_CORES)), trace=trace, **kw
    )
    raw = np.concatenate(
        [np.asarray(res.results[c]["ctx_out"]) for c in range(N_CORES)], axis=0
    ).astype(np.float32)  # [B, 4, D] partial contexts per column group
    sums = np.concatenate(
        [np.asarray(res.results[c]["colsums"]) for c in range(N_CORES)], axis=0
    ).astype(np.float32)
    out = raw.sum(axis=1) / sums.sum(axis=1, keepdims=True)
    return out, res


def kernel(query, values, W1, b1, W2, b2, V, bv):
    in_maps = _prep_inputs(query, values, W1, b1, W2, b2, V, bv)
    out, _ = _run(in_maps, trace=False)
    return out


# revision 10
# speedup vs baseline: 1.2120x; 1.0019x over previous
"""Bahdanau attention kernel for Trainium2 (8 NeuronCores, data-parallel over batch).

Reference computation (B=32, T=4096, D=U=512):
    q_proj = query @ W1 + b1                      [B, 1, U]
    v_proj = values @ W2 + b2                     [B, T, U]
    scores = tanh(q_proj + v_proj) @ V + bv       [B, T, 1]
    attn   = softmax(scores, axis=1)
    out    = sum(attn * values, axis=1)           [B, D]

Device strategy (per core, 4 batches), using only PE + ACT + DMA:
  - Host folds b1/b2 into q_eff = query@W1 + b1 + b2, drops bv (softmax shift
    invariance), ships values twice: natural [T, D] bf16 (context matmul) and
    transposed [D, T] fp8 (projection matmul). All DRAM tensors are
    pre-swizzled on the host into the exact SBUF tile layout so every DMA
    reads large contiguous per-partition segments (max-size packets).
  - v_proj computed transposed [U, t] with W2 stationary; fp8 DoubleRow
    (2 matmuls of K=256) with W2 pre-scaled by F8_SCALE on host, un-scaled
    inside the ACT tanh (scale=1/F8_SCALE); q_eff rides the tanh bias.
  - scores: tanh tiles become the stationary operand against V [128,1], so
    scores land directly in [128, T/128] partition-major PSUM layout. The
    stationary loads run at FWL 4x rate, ~25ns per 128-col tile.
  - softmax without division or max-subtraction (|scores| <= ~1.3 here):
    attn = exp(s); normalization by sum happens on the host via colsums.
  - Context: 4-way column-tiled accumulating [128,1]x[128,512] matmuls:
    t-block n goes to PE column-group n%4 (tile_position=(0,32*(n%4))),
    4 matmuls stream concurrently through disjoint column strips. Partial
    contexts land on PSUM partitions {0,32,64,96}; host sums the 4 rows.
"""

import os
import sys

import numpy as np

try:
    import ml_dtypes  # noqa: F401
except ImportError:  # pragma: no cover
    sys.path.insert(0, "/opt/trn_rl_repo")
    import ml_dtypes  # noqa: F401

try:
    import concourse  # noqa: F401
except ImportError:  # pragma: no cover
    sys.path.insert(0, "/opt/trn_rl_repo")

BF16 = np.dtype(ml_dtypes.bfloat16)
FP8 = np.dtype(ml_dtypes.float8_e4m3)

B, T, D, U = 32, 4096, 512, 512
N_CORES = 8
BPC = B // N_CORES  # batches per core

F8_SCALE = 64.0  # host scales W2 by this; ACT tanh un-scales via scale=1/F8_SCALE

MODE = os.environ.get("BAHDANAU_MODE", "fp8")  # "fp8" | "bf16"

_MODULES: dict = {}


def _build(bpc: int = BPC, t: int = T, mode: str = "fp8"):
    """Build + compile the per-core Bass module. Shapes are per-core shards."""
    from contextlib import ExitStack

    import concourse.bass as bass
    import concourse.tile as tile
    from concourse import bacc, mybir

    f32 = mybir.dt.float32
    bf16 = mybir.dt.bfloat16
    fp8 = mybir.dt.float8e4
    FT = mybir.ActivationFunctionType
    PSUM = bass.MemorySpace.PSUM
    DR = mybir.MatmulPerfMode.DoubleRow

    use_fp8 = mode == "fp8"
    vt_dt = fp8 if use_fp8 else bf16
    tb_n = t // 128  # 128-row t-blocks per batch (32)
    tc_n = t // 512  # 512-col t-chunks per batch (8)
    ch_n = t // 1024  # 1024-col DMA chunks per batch (4)
    tanh_scale = (1.0 / F8_SCALE) if use_fp8 else 1.0

    nc = bacc.Bacc(
        "TRN2", target_bir_lowering=False, debug=False, enable_asserts=False
    )

    # All DRAM layouts match the SBUF tile layouts exactly (host pre-swizzles)
    vT_d = nc.dram_tensor("valuesT", [bpc, ch_n, 128, 4, 1024], vt_dt,
                          kind="ExternalInput")
    vN_d = nc.dram_tensor("valuesN", [bpc, 4, 128, 8, 512], bf16,
                          kind="ExternalInput")
    w2_d = nc.dram_tensor("w2t", [128, 4, U], vt_dt, kind="ExternalInput")
    vc_d = nc.dram_tensor("v_col", [128, 4], bf16, kind="ExternalInput")
    qe_d = nc.dram_tensor("q_eff", [128, bpc, 4], f32, kind="ExternalInput")
    c1b_d = nc.dram_tensor("c_ones_bf", [128, 1], bf16, kind="ExternalInput")
    out_d = nc.dram_tensor("ctx_out", [bpc, 4, D], f32, kind="ExternalOutput")
    cols_d = nc.dram_tensor("colsums", [bpc, tb_n], f32, kind="ExternalOutput")

    with tile.TileContext(nc) as tc, ExitStack() as ctx:
        const = ctx.enter_context(tc.tile_pool(name="const", bufs=1))
        vT_pool = ctx.enter_context(tc.tile_pool(name="vT", bufs=3))
        vN_pool = ctx.enter_context(tc.tile_pool(name="vN", bufs=3))
        tanh_pool = ctx.enter_context(tc.tile_pool(name="tanh", bufs=8))
        sm_pool = ctx.enter_context(tc.tile_pool(name="sm", bufs=2))
        attn_pool = ctx.enter_context(tc.tile_pool(name="attn", bufs=2))
        ctxs_pool = ctx.enter_context(tc.tile_pool(name="ctxs", bufs=2))
        vp_psum = ctx.enter_context(tc.tile_pool(name="vp_ps", bufs=2, space=PSUM))
        sco_psum = ctx.enter_context(tc.tile_pool(name="sc_ps", bufs=2, space=PSUM))
        ctx_psum = ctx.enter_context(tc.tile_pool(name="ctx_ps", bufs=1, space=PSUM))
        sms_psum = ctx.enter_context(tc.tile_pool(name="sm_ps", bufs=1, space=PSUM))

        # Startup order matters: w2 + the first vT half-chunk gate the first
        # projection matmul, qe gates the first tanh. Everything else follows.
        w2_sb = const.tile([128, 4, U], vt_dt)
        nc.sync.dma_start(w2_sb[:, :2], w2_d.ap()[:, :2])
        vT0_sb = vT_pool.tile([128, ch_n, 4, 1024], vt_dt, tag="vT")
        nc.sync.dma_start(vT0_sb[:, 0, :, :512], vT_d[0, 0][:, :, :512])
        nc.sync.dma_start(w2_sb[:, 2:], w2_d.ap()[:, 2:])
        qe_sb = const.tile([128, bpc, 4], f32)
        nc.sync.dma_start(qe_sb[:], qe_d.ap())
        nc.sync.dma_start(vT0_sb[:, 0, :, 512:], vT_d[0, 0][:, :, 512:])
        vc_sb = const.tile([128, 4], bf16)
        nc.sync.dma_start(vc_sb[:], vc_d.ap())
        c1b_sb = const.tile([128, 1], bf16)
        nc.sync.dma_start(c1b_sb[:], c1b_d.ap())

        def stage(b, prev_tail):
            """Full per-batch pipeline: load, project, scores, exp, context.

            The previous batch's final-pair context/sum work (`prev_tail`) is
            emitted after this batch's first pair so the PE never stalls the
            ACT pipeline at batch boundaries. Returns this batch's tail."""
            n_pairs = tc_n // 2
            if b == 0:
                vT_sb = vT0_sb  # chunk 0 already in flight from the preamble
                for c in range(1, ch_n):
                    nc.sync.dma_start(vT_sb[:, c], vT_d[b, c])
            else:
                vT_sb = vT_pool.tile([128, ch_n, 4, 1024], vt_dt, tag="vT")
                for c in range(ch_n):
                    nc.sync.dma_start(vT_sb[:, c], vT_d[b, c])
            vN_sb = vN_pool.tile([128, 4, 8, D], bf16)
            scoresP = sco_psum.tile([128, tb_n], f32)
            expP = attn_pool.tile([128, tb_n], bf16)
            cps = ctx_psum.tile([128, D], f32)

            def ctx_mms(pair):
                # 4-way column-tiled accumulation: t-block n -> col group n%4,
                # 4 concurrent matmuls in disjoint column strips. Each group's
                # first matmul carries start=True (clears has_written for its
                # own partition region) and its last carries stop=True.
                for k in range(8):
                    n = pair * 8 + k
                    g = n % 4
                    nc.tensor.matmul(
                        cps[32 * g : 32 * g + 1, :],
                        expP[:, n : n + 1],
                        vN_sb[:, pair, k, :],
                        start=(n < 4),
                        stop=(n >= tb_n - 4),
                        tile_position=(0, 32 * g),
                        skip_group_check=True,
                    )

            for pair in range(n_pairs):
                nc.sync.dma_start(vN_sb[:, pair], vN_d[b, pair])
                tanh_tiles = []
                for ub in range(4):
                    vp = vp_psum.tile([128, 2, 512], f32)
                    # j outer / half inner so consecutive matmuls share the
                    # same stationary W2 block (LDWEIGHTS amortization)
                    if use_fp8:
                        for j in range(2):
                            for half in range(2):
                                tc8 = pair * 2 + half
                                nc.tensor.matmul(
                                    vp[:, half, :],
                                    w2_sb[:, 2 * j : 2 * j + 2, bass.ts(ub, 128)],
                                    vT_sb[:, tc8 // 2, 2 * j : 2 * j + 2,
                                          bass.ts(tc8 % 2, 512)],
                                    start=(j == 0),
                                    stop=(j == 1),
                                    perf_mode=DR,
                                )
                    else:
                        for j in range(4):
                            for half in range(2):
                                tc8 = pair * 2 + half
                                nc.tensor.matmul(
                                    vp[:, half, :],
                                    w2_sb[:, j, bass.ts(ub, 128)],
                                    vT_sb[:, tc8 // 2, j, bass.ts(tc8 % 2, 512)],
                                    start=(j == 0),
                                    stop=(j == 3),
                                )
                    th = tanh_pool.tile([128, 2, 512], bf16)
                    nc.scalar.activation(
                        th[:],
                        vp[:],
                        FT.Tanh,
                        bias=qe_sb[:, b, ub : ub + 1],
                        scale=tanh_scale,
                    )
                    tanh_tiles.append(th)
                # context matmuls for the previous pair run on PE while ACT is
                # still computing this pair's tanh tiles (they only need the
                # previous exp + vN), hiding the PE->ACT pipeline slack before
                # this pair's score matmuls.
                if pair == 0 and prev_tail is not None:
                    prev_tail()
                if pair > 0:
                    ctx_mms(pair - 1)
                for tl8 in range(8):
                    blk = pair * 8 + tl8
                    for ub in range(4):
                        nc.tensor.matmul(
                            scoresP[:, blk : blk + 1],
                            tanh_tiles[ub][:, tl8 // 4, bass.ts(tl8 % 4, 128)],
                            vc_sb[:, ub : ub + 1],
                            start=(ub == 0),
                            stop=(ub == 3),
                        )
                # unnormalized softmax weights for this pair; division by the
                # sum of exp happens on the host via the colsums output
                nc.scalar.activation(
                    expP[:, pair * 8 : (pair + 1) * 8],
                    scoresP[:, pair * 8 : (pair + 1) * 8],
                    FT.Exp,
                )

            def tail():
                ctx_mms(n_pairs - 1)
                ps1 = sms_psum.tile([tb_n, 1], f32, tag="sm")
                nc.tensor.matmul(
                    ps1[:], expP[:], c1b_sb[:], start=True, stop=True
                )
                s1 = sm_pool.tile([tb_n, 1], f32, tag="s1")
                nc.vector.tensor_copy(s1[:], ps1[:])
                nc.sync.dma_start(
                    cols_d[b : b + 1, :].rearrange("one n -> n one"), s1[:]
                )
                cs_raw = ctxs_pool.tile([128, D], f32)
                nc.vector.tensor_copy(cs_raw[:], cps[:])
                nc.sync.dma_start(
                    out_d[b],
                    cs_raw.rearrange("(g s) d -> g s d", s=32)[:, 0, :],
                )

            return tail

        pend = None
        for b in range(bpc):
            pend = stage(b, pend)
        pend()

    nc.compile()
    return nc


def _get_module(bpc: int = BPC, t: int = T, mode: str | None = None):
    mode = MODE if mode is None else mode
    key = (mode, bpc, t)
    if key not in _MODULES:
        _MODULES[key] = _build(bpc, t, mode)
    return _MODULES[key]


def _prep_inputs(query, values, W1, b1, W2, b2, V, bv, mode: str | None = None):
    """Host-side preprocessing: fold biases, cast, transpose, swizzle, shard."""
    mode = MODE if mode is None else mode
    query = np.asarray(query, np.float32)
    values = np.asarray(values, np.float32)
    W1 = np.asarray(W1, np.float32)
    b1 = np.asarray(b1, np.float32)
    W2 = np.asarray(W2, np.float32)
    b2 = np.asarray(b2, np.float32)
    V = np.asarray(V, np.float32)

    q_eff = (
        query.astype(np.float64) @ W1.astype(np.float64)
        + b1.astype(np.float64)
        + b2.astype(np.float64)
    ).astype(np.float32)  # [B, U]; bv dropped (softmax shift invariance)

    # values natural [T, D] -> [4(pair), 128(p), 8(blk), 512] per batch
    vN = np.ascontiguousarray(
        values.reshape(B, 4, 8, 128, D).transpose(0, 1, 3, 2, 4)
    ).astype(BF16)
    # values transposed [D, T] -> [4(chunk), 128(p), 4(db), 1024] per batch
    vTf = values.transpose(0, 2, 1).reshape(B, 4, 128, 4, 1024)
    vTf = np.ascontiguousarray(vTf.transpose(0, 3, 2, 1, 4))
    if mode == "fp8":
        vT = vTf.astype(FP8)
        w2 = (W2 * F8_SCALE).astype(FP8)
    else:
        vT = vTf.astype(BF16)
        w2 = W2.astype(BF16)
    # W2 [D, U] -> [128(p), 4(db), U]
    w2 = np.ascontiguousarray(w2.reshape(4, 128, U).transpose(1, 0, 2))
    # V [U, 1] -> [128(p), 4(ub)]
    vcol = np.ascontiguousarray(V.reshape(4, 128).T).astype(BF16)
    # q_eff [B, U] -> per-core [128(p), bpc, 4(ub)]
    qe = np.ascontiguousarray(
        q_eff.reshape(B, 4, 128).transpose(2, 0, 1)
    )
    c1b = np.ones((128, 1), BF16)

    in_maps = []
    for c in range(N_CORES):
        s = slice(c * BPC, (c + 1) * BPC)
        in_maps.append(
            {
                "valuesT": vT[s],
                "valuesN": vN[s],
                "w2t": w2,
                "v_col": vcol,
                "q_eff": qe[:, s],
                "c_ones_bf": c1b,
            }
        )
    return in_maps


def _run(in_maps, trace=False, mode: str | None = None, **kw):
    from concourse.bass_utils import run_bass_kernel_spmd

    nc = _get_module(mode=mode)
    res = run_bass_kernel_spmd(
        nc, in_maps, core_ids=list(range(N_CORES)), trace=trace, **kw
    )
    raw = np.concatenate(
        [np.asarray(res.results[c]["ctx_out"]) for c in range(N_CORES)], axis=0
    ).astype(np.float32)  # [B, 4, D] partial contexts per column group
    sums = np.concatenate(
        [np.asarray(res.results[c]["colsums"]) for c in range(N_CORES)], axis=0
    ).astype(np.float32)
    out = raw.sum(axis=1) / sums.sum(axis=1, keepdims=True)
    return out, res


def kernel(query, values, W1, b1, W2, b2, V, bv):
    in_maps = _prep_inputs(query, values, W1, b1, W2, b2, V, bv)
    out, _ = _run(in_maps, trace=False)
    return out


# revision 11
# speedup vs baseline: 1.2916x; 1.0657x over previous
"""Bahdanau attention kernel for Trainium2 (8 NeuronCores, data-parallel over batch).

Reference computation (B=32, T=4096, D=U=512):
    q_proj = query @ W1 + b1                      [B, 1, U]
    v_proj = values @ W2 + b2                     [B, T, U]
    scores = tanh(q_proj + v_proj) @ V + bv       [B, T, 1]
    attn   = softmax(scores, axis=1)
    out    = sum(attn * values, axis=1)           [B, D]

Device strategy (per core, 4 batches), using only PE + ACT + DMA:
  - Host folds b1/b2 into q_eff = query@W1 + b1 + b2, drops bv (softmax shift
    invariance), ships values twice: natural [T, D] bf16 (context matmul) and
    transposed [D, T] fp8 (projection matmul). All DRAM tensors are
    pre-swizzled on the host into the exact SBUF tile layout so every DMA
    reads large contiguous per-partition segments (max-size packets).
  - v_proj computed transposed [U, t] with W2 stationary; fp8 DoubleRow
    (2 matmuls of K=256) with W2 pre-scaled by F8_SCALE on host, un-scaled
    inside the ACT tanh (scale=1/F8_SCALE); q_eff rides the tanh bias.
  - scores: tanh tiles become the stationary operand against V [128,1], so
    scores land directly in [128, T/128] partition-major PSUM layout. The
    stationary loads run at FWL 4x rate, ~25ns per 128-col tile.
  - softmax without division or max-subtraction (|scores| <= ~1.3 here):
    attn = exp(s); normalization by sum happens on the host via colsums.
  - Context: 4-way column-tiled accumulating [128,1]x[128,512] matmuls:
    t-block n goes to PE column-group n%4 (tile_position=(0,32*(n%4))),
    4 matmuls stream concurrently through disjoint column strips. Partial
    contexts land on PSUM partitions {0,32,64,96}; host sums the 4 rows.
"""

import os
import sys

import numpy as np

try:
    import ml_dtypes  # noqa: F401
except ImportError:  # pragma: no cover
    sys.path.insert(0, "/opt/trn_rl_repo")
    import ml_dtypes  # noqa: F401

try:
    import concourse  # noqa: F401
except ImportError:  # pragma: no cover
    sys.path.insert(0, "/opt/trn_rl_repo")

BF16 = np.dtype(ml_dtypes.bfloat16)
FP8 = np.dtype(ml_dtypes.float8_e4m3)

B, T, D, U = 32, 4096, 512, 512
N_CORES = 8
BPC = B // N_CORES  # batches per core

F8_SCALE = 64.0  # host scales W2 by this; ACT tanh un-scales via scale=1/F8_SCALE

MODE = os.environ.get("BAHDANAU_MODE", "fp8")  # "fp8" | "bf16"

_MODULES: dict = {}


def _build(bpc: int = BPC, t: int = T, mode: str = "fp8"):
    """Build + compile the per-core Bass module. Shapes are per-core shards."""
    from contextlib import ExitStack

    import concourse.bass as bass
    import concourse.tile as tile
    from concourse import bacc, mybir

    f32 = mybir.dt.float32
    bf16 = mybir.dt.bfloat16
    fp8 = mybir.dt.float8e4
    FT = mybir.ActivationFunctionType
    PSUM = bass.MemorySpace.PSUM
    DR = mybir.MatmulPerfMode.DoubleRow

    use_fp8 = mode == "fp8"
    vt_dt = fp8 if use_fp8 else bf16
    tb_n = t // 128  # 128-row t-blocks per batch (32)
    tc_n = t // 512  # 512-col t-chunks per batch (8)
    ch_n = t // 1024  # 1024-col DMA chunks per batch (4)
    tanh_scale = (1.0 / F8_SCALE) if use_fp8 else 1.0

    nc = bacc.Bacc(
        "TRN2", target_bir_lowering=False, debug=False, enable_asserts=False
    )

    # All DRAM layouts match the SBUF tile layouts exactly (host pre-swizzles)
    vT_d = nc.dram_tensor("valuesT", [bpc, ch_n, 128, 4, 1024], vt_dt,
                          kind="ExternalInput")
    vN_d = nc.dram_tensor("valuesN", [bpc, 4, 128, 8, 512], bf16,
                          kind="ExternalInput")
    w2_d = nc.dram_tensor("w2t", [128, 4, U], vt_dt, kind="ExternalInput")
    vc_d = nc.dram_tensor("v_col", [128, 4], bf16, kind="ExternalInput")
    qe_d = nc.dram_tensor("q_eff", [128, bpc, 4], f32, kind="ExternalInput")
    c1b_d = nc.dram_tensor("c_ones_bf", [128, 1], bf16, kind="ExternalInput")
    out_d = nc.dram_tensor("ctx_out", [bpc, 4, D], f32, kind="ExternalOutput")
    cols_d = nc.dram_tensor("colsums", [bpc, tb_n], f32, kind="ExternalOutput")

    with tile.TileContext(nc) as tc, ExitStack() as ctx:
        const = ctx.enter_context(tc.tile_pool(name="const", bufs=1))
        vT_pool = ctx.enter_context(tc.tile_pool(name="vT", bufs=3))
        vN_pool = ctx.enter_context(tc.tile_pool(name="vN", bufs=3))
        tanh_pool = ctx.enter_context(tc.tile_pool(name="tanh", bufs=8))
        sm_pool = ctx.enter_context(tc.tile_pool(name="sm", bufs=2))
        attn_pool = ctx.enter_context(tc.tile_pool(name="attn", bufs=2))
        ctxs_pool = ctx.enter_context(tc.tile_pool(name="ctxs", bufs=2))
        vp_psum = ctx.enter_context(tc.tile_pool(name="vp_ps", bufs=2, space=PSUM))
        sco_psum = ctx.enter_context(tc.tile_pool(name="sc_ps", bufs=2, space=PSUM))
        ctx_psum = ctx.enter_context(tc.tile_pool(name="ctx_ps", bufs=1, space=PSUM))
        sms_psum = ctx.enter_context(tc.tile_pool(name="sm_ps", bufs=1, space=PSUM))

        # Startup order matters: w2 + the first vT half-chunk gate the first
        # projection matmul, qe gates the first tanh. Everything else follows.
        w2_sb = const.tile([128, 4, U], vt_dt)
        nc.sync.dma_start(w2_sb[:, :2], w2_d.ap()[:, :2])
        vT0_sb = vT_pool.tile([128, ch_n, 4, 1024], vt_dt, tag="vT")
        nc.sync.dma_start(vT0_sb[:, 0, :, :512], vT_d[0, 0][:, :, :512])
        nc.sync.dma_start(w2_sb[:, 2:], w2_d.ap()[:, 2:])
        qe_sb = const.tile([128, bpc, 4], f32)
        nc.sync.dma_start(qe_sb[:], qe_d.ap())
        nc.sync.dma_start(vT0_sb[:, 0, :, 512:], vT_d[0, 0][:, :, 512:])
        vc_sb = const.tile([128, 4], bf16)
        nc.sync.dma_start(vc_sb[:], vc_d.ap())
        c1b_sb = const.tile([128, 1], bf16)
        nc.sync.dma_start(c1b_sb[:], c1b_d.ap())

        def stage(b, prev_tail):
            """Full per-batch pipeline: load, project, scores, exp, context.

            The previous batch's final-pair context/sum work (`prev_tail`) is
            emitted after this batch's first pair so the PE never stalls the
            ACT pipeline at batch boundaries. Returns this batch's tail."""
            n_pairs = tc_n // 2
            if b == 0:
                vT_sb = vT0_sb  # chunk 0 already in flight from the preamble
                for c in range(1, ch_n):
                    nc.sync.dma_start(vT_sb[:, c], vT_d[b, c])
            else:
                vT_sb = vT_pool.tile([128, ch_n, 4, 1024], vt_dt, tag="vT")
                for c in range(ch_n):
                    nc.sync.dma_start(vT_sb[:, c], vT_d[b, c])
            vN_sb = vN_pool.tile([128, 4, 8, D], bf16)
            scoresP = sco_psum.tile([128, tb_n], f32)
            expP = attn_pool.tile([128, tb_n], bf16)
            cps = ctx_psum.tile([128, D], f32)

            def ctx_mms(pair):
                # 4-way column-tiled accumulation: t-block n -> col group n%4,
                # 4 concurrent matmuls in disjoint column strips. Each group's
                # first matmul carries start=True (clears has_written for its
                # own partition region) and its last carries stop=True.
                for k in range(8):
                    n = pair * 8 + k
                    g = n % 4
                    nc.tensor.matmul(
                        cps[32 * g : 32 * g + 1, :],
                        expP[:, n : n + 1],
                        vN_sb[:, pair, k, :],
                        start=(n < 4),
                        stop=(n >= tb_n - 4),
                        tile_position=(0, 32 * g),
                        skip_group_check=True,
                    )

            for pair in range(n_pairs):
                nc.sync.dma_start(vN_sb[:, pair], vN_d[b, pair])
                tanh_tiles = []
                for ub in range(4):
                    vp = vp_psum.tile([128, 2, 512], f32)
                    # j outer / half inner so consecutive matmuls share the
                    # same stationary W2 block (LDWEIGHTS amortization)
                    if use_fp8:
                        for j in range(2):
                            for half in range(2):
                                tc8 = pair * 2 + half
                                nc.tensor.matmul(
                                    vp[:, half, :],
                                    w2_sb[:, 2 * j : 2 * j + 2, bass.ts(ub, 128)],
                                    vT_sb[:, tc8 // 2, 2 * j : 2 * j + 2,
                                          bass.ts(tc8 % 2, 512)],
                                    start=(j == 0),
                                    stop=(j == 1),
                                    perf_mode=DR,
                                )
                    else:
                        for j in range(4):
                            for half in range(2):
                                tc8 = pair * 2 + half
                                nc.tensor.matmul(
                                    vp[:, half, :],
                                    w2_sb[:, j, bass.ts(ub, 128)],
                                    vT_sb[:, tc8 // 2, j, bass.ts(tc8 % 2, 512)],
                                    start=(j == 0),
                                    stop=(j == 3),
                                )
                    th = tanh_pool.tile([128, 2, 512], bf16)
                    nc.scalar.activation(
                        th[:],
                        vp[:],
                        FT.Tanh,
                        bias=qe_sb[:, b, ub : ub + 1],
                        scale=tanh_scale,
                    )
                    tanh_tiles.append(th)
                # context matmuls for the previous pair run on PE while ACT is
                # still computing this pair's tanh tiles (they only need the
                # previous exp + vN), hiding the PE->ACT pipeline slack before
                # this pair's score matmuls.
                if pair == 0 and prev_tail is not None:
                    prev_tail()
                if pair > 0:
                    ctx_mms(pair - 1)
                # ub outer: the ub=0 score pass only needs the first tanh tile,
                # so PE pipelines against ACT instead of waiting for all four
                for ub in range(4):
                    for tl8 in range(8):
                        blk = pair * 8 + tl8
                        nc.tensor.matmul(
                            scoresP[:, blk : blk + 1],
                            tanh_tiles[ub][:, tl8 // 4, bass.ts(tl8 % 4, 128)],
                            vc_sb[:, ub : ub + 1],
                            start=(ub == 0),
                            stop=(ub == 3),
                        )
                # unnormalized softmax weights for this pair; division by the
                # sum of exp happens on the host via the colsums output
                nc.scalar.activation(
                    expP[:, pair * 8 : (pair + 1) * 8],
                    scoresP[:, pair * 8 : (pair + 1) * 8],
                    FT.Exp,
                )

            def tail():
                ctx_mms(n_pairs - 1)
                ps1 = sms_psum.tile([tb_n, 1], f32, tag="sm")
                nc.tensor.matmul(
                    ps1[:], expP[:], c1b_sb[:], start=True, stop=True
                )
                s1 = sm_pool.tile([tb_n, 1], f32, tag="s1")
                nc.vector.tensor_copy(s1[:], ps1[:])
                nc.sync.dma_start(
                    cols_d[b : b + 1, :].rearrange("one n -> n one"), s1[:]
                )
                cs_raw = ctxs_pool.tile([128, D], f32)
                nc.vector.tensor_copy(cs_raw[:], cps[:])
                nc.sync.dma_start(
                    out_d[b],
                    cs_raw.rearrange("(g s) d -> g s d", s=32)[:, 0, :],
                )

            return tail

        pend = None
        for b in range(bpc):
            pend = stage(b, pend)
        pend()

    nc.compile()
    return nc


def _get_module(bpc: int = BPC, t: int = T, mode: str | None = None):
    mode = MODE if mode is None else mode
    key = (mode, bpc, t)
    if key not in _MODULES:
        _MODULES[key] = _build(bpc, t, mode)
    return _MODULES[key]


def _prep_inputs(query, values, W1, b1, W2, b2, V, bv, mode: str | None = None):
    """Host-side preprocessing: fold biases, cast, transpose, swizzle, shard."""
    mode = MODE if mode is None else mode
    query = np.asarray(query, np.float32)
    values = np.asarray(values, np.float32)
    W1 = np.asarray(W1, np.float32)
    b1 = np.asarray(b1, np.float32)
    W2 = np.asarray(W2, np.float32)
    b2 = np.asarray(b2, np.float32)
    V = np.asarray(V, np.float32)

    q_eff = (
        query.astype(np.float64) @ W1.astype(np.float64)
        + b1.astype(np.float64)
        + b2.astype(np.float64)
    ).astype(np.float32)  # [B, U]; bv dropped (softmax shift invariance)

    # values natural [T, D] -> [4(pair), 128(p), 8(blk), 512] per batch
    vN = np.ascontiguousarray(
        values.reshape(B, 4, 8, 128, D).transpose(0, 1, 3, 2, 4)
    ).astype(BF16)
    # values transposed [D, T] -> [4(chunk), 128(p), 4(db), 1024] per batch
    vTf = values.transpose(0, 2, 1).reshape(B, 4, 128, 4, 1024)
    vTf = np.ascontiguousarray(vTf.transpose(0, 3, 2, 1, 4))
    if mode == "fp8":
        vT = vTf.astype(FP8)
        w2 = (W2 * F8_SCALE).astype(FP8)
    else:
        vT = vTf.astype(BF16)
        w2 = W2.astype(BF16)
    # W2 [D, U] -> [128(p), 4(db), U]
    w2 = np.ascontiguousarray(w2.reshape(4, 128, U).transpose(1, 0, 2))
    # V [U, 1] -> [128(p), 4(ub)]
    vcol = np.ascontiguousarray(V.reshape(4, 128).T).astype(BF16)
    # q_eff [B, U] -> per-core [128(p), bpc, 4(ub)]
    qe = np.ascontiguousarray(
        q_eff.reshape(B, 4, 128).transpose(2, 0, 1)
    )
    c1b = np.ones((128, 1), BF16)

    in_maps = []
    for c in range(N_CORES):
        s = slice(c * BPC, (c + 1) * BPC)
        in_maps.append(
            {
                "valuesT": vT[s],
                "valuesN": vN[s],
                "w2t": w2,
                "v_col": vcol,
                "q_eff": qe[:, s],
                "c_ones_bf": c1b,
            }
        )
    return in_maps


def _run(in_maps, trace=False, mode: str | None = None, **kw):
    from concourse.bass_utils import run_bass_kernel_spmd

    nc = _get_module(mode=mode)
    res = run_bass_kernel_spmd(
        nc, in_maps, core_ids=list(range(N_CORES)), trace=trace, **kw
    )
    raw = np.concatenate(
        [np.asarray(res.results[c]["ctx_out"]) for c in range(N_CORES)], axis=0
    ).astype(np.float32)  # [B, 4, D] partial contexts per column group
    sums = np.concatenate(
        [np.asarray(res.results[c]["colsums"]) for c in range(N_CORES)], axis=0
    ).astype(np.float32)
    out = raw.sum(axis=1) / sums.sum(axis=1, keepdims=True)
    return out, res


def kernel(query, values, W1, b1, W2, b2, V, bv):
    in_maps = _prep_inputs(query, values, W1, b1, W2, b2, V, bv)
    out, _ = _run(in_maps, trace=False)
    return out


# revision 12
# speedup vs baseline: 1.2997x; 1.0063x over previous
"""Bahdanau attention kernel for Trainium2 (8 NeuronCores, data-parallel over batch).

Reference computation (B=32, T=4096, D=U=512):
    q_proj = query @ W1 + b1                      [B, 1, U]
    v_proj = values @ W2 + b2                     [B, T, U]
    scores = tanh(q_proj + v_proj) @ V + bv       [B, T, 1]
    attn   = softmax(scores, axis=1)
    out    = sum(attn * values, axis=1)           [B, D]

Device strategy (per core, 4 batches), using only PE + ACT + DMA:
  - Host folds b1/b2 into q_eff = query@W1 + b1 + b2, drops bv (softmax shift
    invariance), ships values twice: natural [T, D] bf16 (context matmul) and
    transposed [D, T] fp8 (projection matmul). All DRAM tensors are
    pre-swizzled on the host into the exact SBUF tile layout so every DMA
    reads large contiguous per-partition segments (max-size packets).
  - v_proj computed transposed [U, t] with W2 stationary; fp8 DoubleRow
    (2 matmuls of K=256) with W2 pre-scaled by F8_SCALE on host, un-scaled
    inside the ACT tanh (scale=1/F8_SCALE); q_eff rides the tanh bias.
  - scores: tanh tiles become the stationary operand against V [128,1], so
    scores land directly in [128, T/128] partition-major PSUM layout. The
    stationary loads run at FWL 4x rate, ~25ns per 128-col tile.
  - softmax without division or max-subtraction (|scores| <= ~1.3 here):
    attn = exp(s); normalization by sum happens on the host via colsums.
  - Context: 4-way column-tiled accumulating [128,1]x[128,512] matmuls:
    t-block n goes to PE column-group n%4 (tile_position=(0,32*(n%4))),
    4 matmuls stream concurrently through disjoint column strips. Partial
    contexts land on PSUM partitions {0,32,64,96}; host sums the 4 rows.
"""

import os
import sys

import numpy as np

try:
    import ml_dtypes  # noqa: F401
except ImportError:  # pragma: no cover
    sys.path.insert(0, "/opt/trn_rl_repo")
    import ml_dtypes  # noqa: F401

try:
    import concourse  # noqa: F401
except ImportError:  # pragma: no cover
    sys.path.insert(0, "/opt/trn_rl_repo")

BF16 = np.dtype(ml_dtypes.bfloat16)
FP8 = np.dtype(ml_dtypes.float8_e4m3)

B, T, D, U = 32, 4096, 512, 512
N_CORES = 8
BPC = B // N_CORES  # batches per core

F8_SCALE = 64.0  # host scales W2 by this; ACT tanh un-scales via scale=1/F8_SCALE

MODE = os.environ.get("BAHDANAU_MODE", "fp8")  # "fp8" | "bf16"

_MODULES: dict = {}


def _build(bpc: int = BPC, t: int = T, mode: str = "fp8"):
    """Build + compile the per-core Bass module. Shapes are per-core shards."""
    from contextlib import ExitStack

    import concourse.bass as bass
    import concourse.tile as tile
    from concourse import bacc, mybir

    f32 = mybir.dt.float32
    bf16 = mybir.dt.bfloat16
    fp8 = mybir.dt.float8e4
    FT = mybir.ActivationFunctionType
    PSUM = bass.MemorySpace.PSUM
    DR = mybir.MatmulPerfMode.DoubleRow

    use_fp8 = mode == "fp8"
    vt_dt = fp8 if use_fp8 else bf16
    tb_n = t // 128  # 128-row t-blocks per batch (32)
    tc_n = t // 512  # 512-col t-chunks per batch (8)
    ch_n = t // 1024  # 1024-col DMA chunks per batch (4)
    tanh_scale = (1.0 / F8_SCALE) if use_fp8 else 1.0

    nc = bacc.Bacc(
        "TRN2", target_bir_lowering=False, debug=False, enable_asserts=False
    )

    # All DRAM layouts match the SBUF tile layouts exactly (host pre-swizzles)
    vT_d = nc.dram_tensor("valuesT", [bpc, ch_n, 128, 4, 1024], vt_dt,
                          kind="ExternalInput")
    vN_d = nc.dram_tensor("valuesN", [bpc, 4, 128, 8, 512], bf16,
                          kind="ExternalInput")
    w2_d = nc.dram_tensor("w2t", [128, 4, U], vt_dt, kind="ExternalInput")
    vc_d = nc.dram_tensor("v_col", [128, 4], bf16, kind="ExternalInput")
    qe_d = nc.dram_tensor("q_eff", [128, bpc, 4], f32, kind="ExternalInput")
    c1b_d = nc.dram_tensor("c_ones_bf", [128, 1], bf16, kind="ExternalInput")
    out_d = nc.dram_tensor("ctx_out", [bpc, 4, D], f32, kind="ExternalOutput")
    cols_d = nc.dram_tensor("colsums", [bpc, tb_n], f32, kind="ExternalOutput")

    with tile.TileContext(nc) as tc, ExitStack() as ctx:
        const = ctx.enter_context(tc.tile_pool(name="const", bufs=1))
        vT_pool = ctx.enter_context(tc.tile_pool(name="vT", bufs=3))
        vN_pool = ctx.enter_context(tc.tile_pool(name="vN", bufs=3))
        tanh_pool = ctx.enter_context(tc.tile_pool(name="tanh", bufs=8))
        sm_pool = ctx.enter_context(tc.tile_pool(name="sm", bufs=2))
        attn_pool = ctx.enter_context(tc.tile_pool(name="attn", bufs=2))
        ctxs_pool = ctx.enter_context(tc.tile_pool(name="ctxs", bufs=2))
        vp_psum = ctx.enter_context(tc.tile_pool(name="vp_ps", bufs=2, space=PSUM))
        sco_psum = ctx.enter_context(tc.tile_pool(name="sc_ps", bufs=2, space=PSUM))
        ctx_psum = ctx.enter_context(tc.tile_pool(name="ctx_ps", bufs=1, space=PSUM))
        sms_psum = ctx.enter_context(tc.tile_pool(name="sm_ps", bufs=1, space=PSUM))

        # Startup order matters: w2 + the first vT half-chunk gate the first
        # projection matmul, qe gates the first tanh. Everything else follows.
        w2_sb = const.tile([128, 4, U], vt_dt)
        nc.sync.dma_start(w2_sb[:, :2], w2_d.ap()[:, :2])
        vT0_sb = vT_pool.tile([128, ch_n, 4, 1024], vt_dt, tag="vT")
        nc.sync.dma_start(vT0_sb[:, 0, :, :512], vT_d[0, 0][:, :, :512])
        nc.sync.dma_start(w2_sb[:, 2:], w2_d.ap()[:, 2:])
        qe_sb = const.tile([128, bpc, 4], f32)
        nc.sync.dma_start(qe_sb[:], qe_d.ap())
        nc.sync.dma_start(vT0_sb[:, 0, :, 512:], vT_d[0, 0][:, :, 512:])
        vc_sb = const.tile([128, 4], bf16)
        nc.sync.dma_start(vc_sb[:], vc_d.ap())
        c1b_sb = const.tile([128, 1], bf16)
        nc.sync.dma_start(c1b_sb[:], c1b_d.ap())

        def stage(b, prev_tail):
            """Full per-batch pipeline: load, project, scores, exp, context.

            The previous batch's final-pair context/sum work (`prev_tail`) is
            emitted after this batch's first pair so the PE never stalls the
            ACT pipeline at batch boundaries. Returns this batch's tail."""
            n_pairs = tc_n // 2
            if b == 0:
                vT_sb = vT0_sb  # chunk 0 already in flight from the preamble
                for c in range(1, ch_n):
                    nc.sync.dma_start(vT_sb[:, c], vT_d[b, c])
            else:
                vT_sb = vT_pool.tile([128, ch_n, 4, 1024], vt_dt, tag="vT")
                for c in range(ch_n):
                    nc.sync.dma_start(vT_sb[:, c], vT_d[b, c])
            vN_sb = vN_pool.tile([128, 4, 8, D], bf16)
            scoresP = sco_psum.tile([128, tb_n], f32)
            expP = attn_pool.tile([128, tb_n], bf16)
            cps = ctx_psum.tile([128, D], f32)

            def ctx_mms(pair):
                # 4-way column-tiled accumulation: t-block n -> col group n%4,
                # 4 concurrent matmuls in disjoint column strips. Each group's
                # first matmul carries start=True (clears has_written for its
                # own partition region) and its last carries stop=True.
                for k in range(8):
                    n = pair * 8 + k
                    g = n % 4
                    nc.tensor.matmul(
                        cps[32 * g : 32 * g + 1, :],
                        expP[:, n : n + 1],
                        vN_sb[:, pair, k, :],
                        start=(n < 4),
                        stop=(n >= tb_n - 4),
                        tile_position=(0, 32 * g),
                        skip_group_check=True,
                    )

            for pair in range(n_pairs):
                nc.sync.dma_start(vN_sb[:, pair], vN_d[b, pair])
                tanh_tiles = []
                for ub in range(4):
                    vp = vp_psum.tile([128, 2, 512], f32)
                    # j outer / half inner so consecutive matmuls share the
                    # same stationary W2 block (LDWEIGHTS amortization)
                    if use_fp8:
                        for j in range(2):
                            for half in range(2):
                                tc8 = pair * 2 + half
                                nc.tensor.matmul(
                                    vp[:, half, :],
                                    w2_sb[:, 2 * j : 2 * j + 2, bass.ts(ub, 128)],
                                    vT_sb[:, tc8 // 2, 2 * j : 2 * j + 2,
                                          bass.ts(tc8 % 2, 512)],
                                    start=(j == 0),
                                    stop=(j == 1),
                                    perf_mode=DR,
                                )
                    else:
                        for j in range(4):
                            for half in range(2):
                                tc8 = pair * 2 + half
                                nc.tensor.matmul(
                                    vp[:, half, :],
                                    w2_sb[:, j, bass.ts(ub, 128)],
                                    vT_sb[:, tc8 // 2, j, bass.ts(tc8 % 2, 512)],
                                    start=(j == 0),
                                    stop=(j == 3),
                                )
                    th = tanh_pool.tile([128, 2, 512], bf16)
                    nc.scalar.activation(
                        th[:],
                        vp[:],
                        FT.Tanh,
                        bias=qe_sb[:, b, ub : ub + 1],
                        scale=tanh_scale,
                    )
                    tanh_tiles.append(th)
                # context matmuls for the previous pair run on PE while ACT is
                # still computing this pair's tanh tiles (they only need the
                # previous exp + vN), hiding the PE->ACT pipeline slack before
                # this pair's score matmuls.
                if pair == 0 and prev_tail is not None:
                    prev_tail()
                if pair > 0:
                    ctx_mms(pair - 1)
                # ub outer: the ub=0 score pass only needs the first tanh tile,
                # so PE pipelines against ACT instead of waiting for all four.
                # start=True only on the batch's very first score matmul: the
                # hardware clears has_written for the whole bank (all columns)
                # on start, so per-block starts would wipe sibling partials;
                # with bits clear the first touch of each column overwrites.
                for ub in range(4):
                    for tl8 in range(8):
                        blk = pair * 8 + tl8
                        nc.tensor.matmul(
                            scoresP[:, blk : blk + 1],
                            tanh_tiles[ub][:, tl8 // 4, bass.ts(tl8 % 4, 128)],
                            vc_sb[:, ub : ub + 1],
                            start=(pair == 0 and ub == 0 and tl8 == 0),
                            stop=(ub == 3),
                            skip_group_check=True,
                        )
                # unnormalized softmax weights for this pair; division by the
                # sum of exp happens on the host via the colsums output
                nc.scalar.activation(
                    expP[:, pair * 8 : (pair + 1) * 8],
                    scoresP[:, pair * 8 : (pair + 1) * 8],
                    FT.Exp,
                )

            def tail():
                ctx_mms(n_pairs - 1)
                ps1 = sms_psum.tile([tb_n, 1], f32, tag="sm")
                nc.tensor.matmul(
                    ps1[:], expP[:], c1b_sb[:], start=True, stop=True
                )
                s1 = sm_pool.tile([tb_n, 1], f32, tag="s1")
                nc.vector.tensor_copy(s1[:], ps1[:])
                nc.sync.dma_start(
                    cols_d[b : b + 1, :].rearrange("one n -> n one"), s1[:]
                )
                cs_raw = ctxs_pool.tile([128, D], f32)
                nc.vector.tensor_copy(cs_raw[:], cps[:])
                nc.sync.dma_start(
                    out_d[b],
                    cs_raw.rearrange("(g s) d -> g s d", s=32)[:, 0, :],
                )

            return tail

        pend = None
        for b in range(bpc):
            pend = stage(b, pend)
        pend()

    nc.compile()
    return nc


def _get_module(bpc: int = BPC, t: int = T, mode: str | None = None):
    mode = MODE if mode is None else mode
    key = (mode, bpc, t)
    if key not in _MODULES:
        _MODULES[key] = _build(bpc, t, mode)
    return _MODULES[key]


def _prep_inputs(query, values, W1, b1, W2, b2, V, bv, mode: str | None = None):
    """Host-side preprocessing: fold biases, cast, transpose, swizzle, shard."""
    mode = MODE if mode is None else mode
    query = np.asarray(query, np.float32)
    values = np.asarray(values, np.float32)
    W1 = np.asarray(W1, np.float32)
    b1 = np.asarray(b1, np.float32)
    W2 = np.asarray(W2, np.float32)
    b2 = np.asarray(b2, np.float32)
    V = np.asarray(V, np.float32)

    q_eff = (
        query.astype(np.float64) @ W1.astype(np.float64)
        + b1.astype(np.float64)
        + b2.astype(np.float64)
    ).astype(np.float32)  # [B, U]; bv dropped (softmax shift invariance)

    # values natural [T, D] -> [4(pair), 128(p), 8(blk), 512] per batch
    vN = np.ascontiguousarray(
        values.reshape(B, 4, 8, 128, D).transpose(0, 1, 3, 2, 4)
    ).astype(BF16)
    # values transposed [D, T] -> [4(chunk), 128(p), 4(db), 1024] per batch
    vTf = values.transpose(0, 2, 1).reshape(B, 4, 128, 4, 1024)
    vTf = np.ascontiguousarray(vTf.transpose(0, 3, 2, 1, 4))
    if mode == "fp8":
        vT = vTf.astype(FP8)
        w2 = (W2 * F8_SCALE).astype(FP8)
    else:
        vT = vTf.astype(BF16)
        w2 = W2.astype(BF16)
    # W2 [D, U] -> [128(p), 4(db), U]
    w2 = np.ascontiguousarray(w2.reshape(4, 128, U).transpose(1, 0, 2))
    # V [U, 1] -> [128(p), 4(ub)]
    vcol = np.ascontiguousarray(V.reshape(4, 128).T).astype(BF16)
    # q_eff [B, U] -> per-core [128(p), bpc, 4(ub)]
    qe = np.ascontiguousarray(
        q_eff.reshape(B, 4, 128).transpose(2, 0, 1)
    )
    c1b = np.ones((128, 1), BF16)

    in_maps = []
    for c in range(N_CORES):
        s = slice(c * BPC, (c + 1) * BPC)
        in_maps.append(
            {
                "valuesT": vT[s],
                "valuesN": vN[s],
                "w2t": w2,
                "v_col": vcol,
                "q_eff": qe[:, s],
                "c_ones_bf": c1b,
            }
        )
    return in_maps


def _run(in_maps, trace=False, mode: str | None = None, **kw):
    from concourse.bass_utils import run_bass_kernel_spmd

    nc = _get_module(mode=mode)
    res = run_bass_kernel_spmd(
        nc, in_maps, core_ids=list(range(N_CORES)), trace=trace, **kw
    )
    raw = np.concatenate(
        [np.asarray(res.results[c]["ctx_out"]) for c in range(N_CORES)], axis=0
    ).astype(np.float32)  # [B, 4, D] partial contexts per column group
    sums = np.concatenate(
        [np.asarray(res.results[c]["colsums"]) for c in range(N_CORES)], axis=0
    ).astype(np.float32)
    out = raw.sum(axis=1) / sums.sum(axis=1, keepdims=True)
    return out, res


def kernel(query, values, W1, b1, W2, b2, V, bv):
    in_maps = _prep_inputs(query, values, W1, b1, W2, b2, V, bv)
    out, _ = _run(in_maps, trace=False)
    return out


# revision 15
# speedup vs baseline: 1.3685x; 1.0529x over previous
"""Bahdanau attention kernel for Trainium2 (8 NeuronCores, data-parallel over batch).

Reference computation (B=32, T=4096, D=U=512):
    q_proj = query @ W1 + b1                      [B, 1, U]
    v_proj = values @ W2 + b2                     [B, T, U]
    scores = tanh(q_proj + v_proj) @ V + bv       [B, T, 1]
    attn   = softmax(scores, axis=1)
    out    = sum(attn * values, axis=1)           [B, D]

Device strategy (per core, 4 batches), using only PE + ACT + DMA:
  - Host folds b1/b2 into q_eff = query@W1 + b1 + b2, drops bv (softmax shift
    invariance), ships values twice: natural [T, D] bf16 (context matmul) and
    transposed [D, T] fp8 (projection matmul). All DRAM tensors are
    pre-swizzled on the host into the exact SBUF tile layout so every DMA
    reads large contiguous per-partition segments (max-size packets).
  - v_proj computed transposed [U, t] with W2 stationary; fp8 DoubleRow
    (2 matmuls of K=256) with W2 pre-scaled by F8_SCALE on host, un-scaled
    inside the ACT tanh (scale=1/F8_SCALE); q_eff rides the tanh bias.
  - scores: tanh tiles become the stationary operand against V [128,1], so
    scores land directly in [128, T/128] partition-major PSUM layout. The
    stationary loads run at FWL 4x rate, ~25ns per 128-col tile.
  - softmax without division or max-subtraction (|scores| <= ~1.3 here):
    attn = exp(s); normalization by sum happens on the host via colsums.
  - Context: 4-way column-tiled accumulating [128,1]x[128,512] matmuls:
    t-block n goes to PE column-group n%4 (tile_position=(0,32*(n%4))),
    4 matmuls stream concurrently through disjoint column strips. Partial
    contexts land on PSUM partitions {0,32,64,96}; host sums the 4 rows.
"""

import os
import sys

import numpy as np

try:
    import ml_dtypes  # noqa: F401
except ImportError:  # pragma: no cover
    sys.path.insert(0, "/opt/trn_rl_repo")
    import ml_dtypes  # noqa: F401

try:
    import concourse  # noqa: F401
except ImportError:  # pragma: no cover
    sys.path.insert(0, "/opt/trn_rl_repo")

BF16 = np.dtype(ml_dtypes.bfloat16)
FP8 = np.dtype(ml_dtypes.float8_e4m3)

B, T, D, U = 32, 4096, 512, 512
N_CORES = 8
BPC = B // N_CORES  # batches per core

F8_SCALE = 64.0  # host scales W2 by this; ACT tanh un-scales via scale=1/F8_SCALE

MODE = os.environ.get("BAHDANAU_MODE", "fp8")  # "fp8" | "bf16"

_MODULES: dict = {}


def _build(bpc: int = BPC, t: int = T, mode: str = "fp8"):
    """Build + compile the per-core Bass module. Shapes are per-core shards."""
    from contextlib import ExitStack

    import concourse.bass as bass
    import concourse.tile as tile
    from concourse import bacc, mybir

    f32 = mybir.dt.float32
    bf16 = mybir.dt.bfloat16
    fp8 = mybir.dt.float8e4
    FT = mybir.ActivationFunctionType
    PSUM = bass.MemorySpace.PSUM
    DR = mybir.MatmulPerfMode.DoubleRow

    use_fp8 = mode == "fp8"
    vt_dt = fp8 if use_fp8 else bf16
    tb_n = t // 128  # 128-row t-blocks per batch (32)
    tc_n = t // 512  # 512-col t-chunks per batch (8)
    ch_n = t // 1024  # 1024-col DMA chunks per batch (4)
    tanh_scale = (1.0 / F8_SCALE) if use_fp8 else 1.0

    nc = bacc.Bacc(
        "TRN2", target_bir_lowering=False, debug=False, enable_asserts=False
    )

    # All DRAM layouts match the SBUF tile layouts exactly (host pre-swizzles)
    vT_d = nc.dram_tensor("valuesT", [bpc, ch_n, 128, 4, 1024], vt_dt,
                          kind="ExternalInput")
    vN_d = nc.dram_tensor("valuesN", [bpc, 4, 128, 8, 512], bf16,
                          kind="ExternalInput")
    w2_d = nc.dram_tensor("w2t", [128, 4, U], vt_dt, kind="ExternalInput")
    vc_d = nc.dram_tensor("v_col", [128, 4], bf16, kind="ExternalInput")
    qe_d = nc.dram_tensor("q_eff", [128, bpc, 4], f32, kind="ExternalInput")
    c1b_d = nc.dram_tensor("c_ones_bf", [128, 1], bf16, kind="ExternalInput")
    out_d = nc.dram_tensor("ctx_out", [bpc, 4, D], f32, kind="ExternalOutput")
    cols_d = nc.dram_tensor("colsums", [bpc, tb_n], f32, kind="ExternalOutput")

    with tile.TileContext(nc) as tc, ExitStack() as ctx:
        const = ctx.enter_context(tc.tile_pool(name="const", bufs=1))
        vT_pool = ctx.enter_context(tc.tile_pool(name="vT", bufs=3))
        vN_pool = ctx.enter_context(tc.tile_pool(name="vN", bufs=3))
        tanh_pool = ctx.enter_context(tc.tile_pool(name="tanh", bufs=8))
        sm_pool = ctx.enter_context(tc.tile_pool(name="sm", bufs=2))
        attn_pool = ctx.enter_context(tc.tile_pool(name="attn", bufs=2))
        ctxs_pool = ctx.enter_context(tc.tile_pool(name="ctxs", bufs=2))
        vp_psum = ctx.enter_context(tc.tile_pool(name="vp_ps", bufs=2, space=PSUM))
        sco_psum = ctx.enter_context(tc.tile_pool(name="sc_ps", bufs=2, space=PSUM))
        ctx_psum = ctx.enter_context(tc.tile_pool(name="ctx_ps", bufs=1, space=PSUM))
        sms_psum = ctx.enter_context(tc.tile_pool(name="sm_ps", bufs=1, space=PSUM))

        # Startup order matters: w2 + the first vT half-chunk gate the first
        # projection matmul, qe gates the first tanh. Everything else follows.
        w2_sb = const.tile([128, 4, U], vt_dt)
        nc.sync.dma_start(w2_sb[:, :2], w2_d.ap()[:, :2])
        vT0_sb = vT_pool.tile([128, ch_n, 4, 1024], vt_dt, tag="vT")
        nc.sync.dma_start(vT0_sb[:, 0, :, :512], vT_d[0, 0][:, :, :512])
        nc.sync.dma_start(w2_sb[:, 2:], w2_d.ap()[:, 2:])
        qe_sb = const.tile([128, bpc, 4], f32)
        nc.sync.dma_start(qe_sb[:], qe_d.ap())
        nc.sync.dma_start(vT0_sb[:, 0, :, 512:], vT_d[0, 0][:, :, 512:])
        vc_sb = const.tile([128, 4], bf16)
        nc.sync.dma_start(vc_sb[:], vc_d.ap())
        c1b_sb = const.tile([128, 1], bf16)
        nc.sync.dma_start(c1b_sb[:], c1b_d.ap())

        def stage(b, prev_tail):
            """Full per-batch pipeline: load, project, scores, exp, context.

            The previous batch's final-pair context/sum work (`prev_tail`) is
            emitted after this batch's first pair so the PE never stalls the
            ACT pipeline at batch boundaries. Returns this batch's tail."""
            n_pairs = tc_n // 2
            if b == 0:
                vT_sb = vT0_sb  # chunk 0 already in flight from the preamble
                for c in range(1, ch_n):
                    nc.sync.dma_start(vT_sb[:, c], vT_d[b, c])
            else:
                vT_sb = vT_pool.tile([128, ch_n, 4, 1024], vt_dt, tag="vT")
                for c in range(ch_n):
                    nc.sync.dma_start(vT_sb[:, c], vT_d[b, c])
            vN_sb = vN_pool.tile([128, 4, 8, D], bf16)
            scoresP = sco_psum.tile([128, tb_n], f32)
            expP = attn_pool.tile([128, tb_n], bf16)
            cps = ctx_psum.tile([128, D], f32)

            def ctx_mms():
                # 4-way column-tiled accumulation: t-block n -> col group n%4,
                # 4 concurrent matmuls in disjoint column strips. Each group's
                # first matmul carries start=True (clears has_written for its
                # own partition region) and its last carries stop=True.
                for n in range(tb_n):
                    g = n % 4
                    nc.tensor.matmul(
                        cps[32 * g : 32 * g + 1, :],
                        expP[:, n : n + 1],
                        vN_sb[:, n // 8, n % 8, :],
                        start=(n < 4),
                        stop=(n >= tb_n - 4),
                        tile_position=(0, 32 * g),
                        skip_group_check=True,
                    )

            for pair in range(n_pairs):
                nc.sync.dma_start(vN_sb[:, pair], vN_d[b, pair])
                tanh_tiles = []
                for ub in range(4):
                    vp = vp_psum.tile([128, 2, 512], f32)
                    # j outer / half inner so consecutive matmuls share the
                    # same stationary W2 block (LDWEIGHTS amortization)
                    if use_fp8:
                        for j in range(2):
                            for half in range(2):
                                tc8 = pair * 2 + half
                                nc.tensor.matmul(
                                    vp[:, half, :],
                                    w2_sb[:, 2 * j : 2 * j + 2, bass.ts(ub, 128)],
                                    vT_sb[:, tc8 // 2, 2 * j : 2 * j + 2,
                                          bass.ts(tc8 % 2, 512)],
                                    start=(j == 0),
                                    stop=(j == 1),
                                    perf_mode=DR,
                                )
                    else:
                        for j in range(4):
                            for half in range(2):
                                tc8 = pair * 2 + half
                                nc.tensor.matmul(
                                    vp[:, half, :],
                                    w2_sb[:, j, bass.ts(ub, 128)],
                                    vT_sb[:, tc8 // 2, j, bass.ts(tc8 % 2, 512)],
                                    start=(j == 0),
                                    stop=(j == 3),
                                )
                    th = tanh_pool.tile([128, 2, 512], bf16)
                    nc.scalar.activation(
                        th[:],
                        vp[:],
                        FT.Tanh,
                        bias=qe_sb[:, b, ub : ub + 1],
                        scale=tanh_scale,
                    )
                    tanh_tiles.append(th)
                # the previous batch's exp + context + sums run here, while ACT
                # is still computing this batch's first tanh tiles, hiding the
                # PE->ACT pipeline slack at the batch boundary
                if pair == 0 and prev_tail is not None:
                    prev_tail()
                # ub outer: the ub=0 score pass only needs the first tanh tile,
                # so PE pipelines against ACT instead of waiting for all four.
                # start=True only on the batch's very first score matmul: the
                # hardware clears has_written for the whole bank (all columns)
                # on start, so per-block starts would wipe sibling partials;
                # with bits clear the first touch of each column overwrites.
                for ub in range(4):
                    for tl8 in range(8):
                        blk = pair * 8 + tl8
                        nc.tensor.matmul(
                            scoresP[:, blk : blk + 1],
                            tanh_tiles[ub][:, tl8 // 4, bass.ts(tl8 % 4, 128)],
                            vc_sb[:, ub : ub + 1],
                            start=(pair == 0 and ub == 0 and tl8 == 0),
                            stop=(ub == 3),
                            skip_group_check=True,
                        )
            def tail():
                # one exp for the whole batch (unnormalized softmax weights;
                # division by the sum of exp happens on the host via colsums),
                # then all 32 context matmuls + the sums matmul
                nc.scalar.activation(expP[:], scoresP[:], FT.Exp)
                ctx_mms()
                ps1 = sms_psum.tile([tb_n, 1], f32, tag="sm")
                nc.tensor.matmul(
                    ps1[:], expP[:], c1b_sb[:], start=True, stop=True
                )
                s1 = sm_pool.tile([tb_n, 1], f32, tag="s1")
                nc.vector.tensor_copy(s1[:], ps1[:])
                nc.sync.dma_start(
                    cols_d[b : b + 1, :].rearrange("one n -> n one"), s1[:]
                )
                cs_raw = ctxs_pool.tile([128, D], f32)
                nc.vector.tensor_copy(cs_raw[:], cps[:])
                nc.sync.dma_start(
                    out_d[b],
                    cs_raw.rearrange("(g s) d -> g s d", s=32)[:, 0, :],
                )

            return tail

        pend = None
        for b in range(bpc):
            pend = stage(b, pend)
        pend()

    nc.compile()
    return nc


def _get_module(bpc: int = BPC, t: int = T, mode: str | None = None):
    mode = MODE if mode is None else mode
    key = (mode, bpc, t)
    if key not in _MODULES:
        _MODULES[key] = _build(bpc, t, mode)
    return _MODULES[key]


def _prep_inputs(query, values, W1, b1, W2, b2, V, bv, mode: str | None = None):
    """Host-side preprocessing: fold biases, cast, transpose, swizzle, shard."""
    mode = MODE if mode is None else mode
    query = np.asarray(query, np.float32)
    values = np.asarray(values, np.float32)
    W1 = np.asarray(W1, np.float32)
    b1 = np.asarray(b1, np.float32)
    W2 = np.asarray(W2, np.float32)
    b2 = np.asarray(b2, np.float32)
    V = np.asarray(V, np.float32)

    q_eff = (
        query.astype(np.float64) @ W1.astype(np.float64)
        + b1.astype(np.float64)
        + b2.astype(np.float64)
    ).astype(np.float32)  # [B, U]; bv dropped (softmax shift invariance)

    # values natural [T, D] -> [4(pair), 128(p), 8(blk), 512] per batch
    vN = np.ascontiguousarray(
        values.reshape(B, 4, 8, 128, D).transpose(0, 1, 3, 2, 4)
    ).astype(BF16)
    # values transposed [D, T] -> [4(chunk), 128(p), 4(db), 1024] per batch
    vTf = values.transpose(0, 2, 1).reshape(B, 4, 128, 4, 1024)
    vTf = np.ascontiguousarray(vTf.transpose(0, 3, 2, 1, 4))
    if mode == "fp8":
        vT = vTf.astype(FP8)
        w2 = (W2 * F8_SCALE).astype(FP8)
    else:
        vT = vTf.astype(BF16)
        w2 = W2.astype(BF16)
    # W2 [D, U] -> [128(p), 4(db), U]
    w2 = np.ascontiguousarray(w2.reshape(4, 128, U).transpose(1, 0, 2))
    # V [U, 1] -> [128(p), 4(ub)]
    vcol = np.ascontiguousarray(V.reshape(4, 128).T).astype(BF16)
    # q_eff [B, U] -> per-core [128(p), bpc, 4(ub)]
    qe = np.ascontiguousarray(
        q_eff.reshape(B, 4, 128).transpose(2, 0, 1)
    )
    c1b = np.ones((128, 1), BF16)

    in_maps = []
    for c in range(N_CORES):
        s = slice(c * BPC, (c + 1) * BPC)
        in_maps.append(
            {
                "valuesT": vT[s],
                "valuesN": vN[s],
                "w2t": w2,
                "v_col": vcol,
                "q_eff": qe[:, s],
                "c_ones_bf": c1b,
            }
        )
    return in_maps


def _run(in_maps, trace=False, mode: str | None = None, **kw):
    from concourse.bass_utils import run_bass_kernel_spmd

    nc = _get_module(mode=mode)
    res = run_bass_kernel_spmd(
        nc, in_maps, core_ids=list(range(N_CORES)), trace=trace, **kw
    )
    raw = np.concatenate(
        [np.asarray(res.results[c]["ctx_out"]) for c in range(N_CORES)], axis=0
    ).astype(np.float32)  # [B, 4, D] partial contexts per column group
    sums = np.concatenate(
        [np.asarray(res.results[c]["colsums"]) for c in range(N_CORES)], axis=0
    ).astype(np.float32)
    out = raw.sum(axis=1) / sums.sum(axis=1, keepdims=True)
    return out, res


def kernel(query, values, W1, b1, W2, b2, V, bv):
    in_maps = _prep_inputs(query, values, W1, b1, W2, b2, V, bv)
    out, _ = _run(in_maps, trace=False)
    return out
